# revision 1
# baseline (speedup 1.0000x reference)
"""Trainium2 Bass kernel for AutoregressiveConvLSTM log-prob.

Strategy
--------
Data-parallel over batch: 64 images -> 8 NeuronCores, 8 images each.

Per-core layout: each (image-batch, channel) "plane" is an SBUF tile
[H=128 partitions, 1042 free] where image b occupies flat columns
130*b+1 .. 130*b+128 and the surrounding columns are zero pads.

All 3x3 convs run on the TensorEngine as banded matmuls:
  out[h_out, col] = sum_h_in Band[h_in, h_out] * plane[h_in, col+dx]
where Band is a [128,128] tri-diagonal matrix holding the three dy taps
(built on the host from the conv weights) and the dx in {-1,0,1} shift
is a free-dim AP offset into the zero pads.  Contributions over
(cin, dx) accumulate in PSUM.  Matmuls use float32r (full fp32 data,
fast PE mode).  LSTM pointwise math runs on ScalarE/VectorE in fp32.

The per-pixel log-prob terms are reduced over W on VectorE into a
[128 (=H), 8 (=image)] accumulator, and over H at the end with a single
ones-vector matmul.  Output per core: [8] -> host concatenates to [64].
"""

import numpy as np

B_FULL, C, H, W, F = 64, 16, 128, 128, 2
NCORES = 8
BL = B_FULL // NCORES            # images per core
WB = W + 2                       # per-image block width incl. pads
FREE = BL * WB + 2               # flat free size (+2 spare zero cols)
HALF_LOG_2PI = 0.9189385332046727

# chunks: (b0, n_imgs, c0, ilo)  — psum columns [c0, c0+n*WB), image b
# starts at local column WB*(b-b0)+ilo, interior slice [ilo, ilo+128)
CHUNKS = [(0, 3, 1, 0), (3, 3, 3 * WB, 1), (6, 2, 6 * WB, 1)]

N_STEP_BANDS = 3 + 72 + 12 + 12          # u, gates, head1, head2
N_ONETIME_BANDS = 96 + 12 + 12           # cond1, cond2, partial1
NB = N_ONETIME_BANDS + N_STEP_BANDS


def _band(w3):
    """[128,128] B[h_in,h_out] = w3[h_in-h_out+1] (tri-diagonal)."""
    b = np.zeros((H, H), np.float32)
    for dy in (-1, 0, 1):
        ar = np.arange(max(0, -dy), H - max(0, dy))
        b[ar + dy, ar] = w3[dy + 1]
    return b


def _build_bands(Wci, Wc1, Wc2, Wo1, Wo2, Wih, Whh):
    bands = np.zeros((NB, H, H), np.float32)
    i = 0
    # one-time: cond1 (16->2, ci-major for group streaming), cond2,
    # partial1 (cond_f part of Wo1)
    for ci in range(16):
        for co in range(2):
            for dx in range(3):
                bands[i] = _band(Wc1[:, dx, ci, co]); i += 1
    for co in range(2):
        for ci in range(2):
            for dx in range(3):
                bands[i] = _band(Wc2[:, dx, ci, co]); i += 1
    for co in range(2):
        for ci in range(2):
            for dx in range(3):
                bands[i] = _band(Wo1[:, dx, 2 + ci, co]); i += 1
    assert i == N_ONETIME_BANDS
    # step bands: u conv (1->1)
    for dx in range(3):
        bands[i] = _band(Wci[:, dx, 0, 0]); i += 1
    # gates: src 0,1 = h planes (Whh), src 2 = u plane (Wih)
    for co in range(8):
        for src in range(3):
            for dx in range(3):
                w3 = Whh[:, dx, src, co] if src < 2 else Wih[:, dx, 0, co]
                bands[i] = _band(w3); i += 1
    # head1 (h part of Wo1), head2 (Wo2)
    for co in range(2):
        for ci in range(2):
            for dx in range(3):
                bands[i] = _band(Wo1[:, dx, ci, co]); i += 1
    for co in range(2):
        for ci in range(2):
            for dx in range(3):
                bands[i] = _band(Wo2[:, dx, ci, co]); i += 1
    assert i == NB
    return bands


def _build_program(bci, bc1, bc2, bo1, bo2, bih):
    import concourse.bacc as bacc
    import concourse.mybir as mybir
    import concourse.tile as tile

    f32 = mybir.dt.float32
    MM = mybir.dt.float32r
    AF = mybir.ActivationFunctionType
    OP = mybir.AluOpType
    AX = mybir.AxisListType

    nc = bacc.Bacc("TRN2", target_bir_lowering=False, debug=False)
    xd = nc.dram_tensor("x", [BL, C, H, W], MM, kind="ExternalInput")
    cd = nc.dram_tensor("cond", [BL, C, H, W], MM, kind="ExternalInput")
    bd = nc.dram_tensor("bands", [NB, H, H], MM, kind="ExternalInput")
    od = nc.dram_tensor("out", [BL, 1], f32, kind="ExternalOutput")

    def i3(ap_flat, b0, n, lo):
        # [128, n, 128] interior view of a [128, >=1040] flat AP
        return ap_flat[:, : BL * WB].rearrange(
            "p (b w) -> p b w", w=WB)[:, b0:b0 + n, lo:lo + 128]

    with tile.TileContext(nc) as tc:
        import contextlib
        ctx = contextlib.ExitStack()
        with ctx:
            state = ctx.enter_context(tc.tile_pool(name="state", bufs=1))
            sbands = ctx.enter_context(tc.tile_pool(name="sbands", bufs=1))
            stream = ctx.enter_context(tc.tile_pool(name="stream", bufs=3))
            ctmp = ctx.enter_context(tc.tile_pool(name="ctmp", bufs=2))
            tmp = ctx.enter_context(tc.tile_pool(name="tmp", bufs=16))
            psum = ctx.enter_context(
                tc.tile_pool(name="psum", bufs=8, space="PSUM"))

            # step bands, resident
            sb = sbands.tile([H, N_STEP_BANDS * H], MM, tag="sb", name="sb")
            for k in range(8):
                s = (N_STEP_BANDS * k) // 8
                e = (N_STEP_BANDS * (k + 1)) // 8
                nc.sync.dma_start(
                    sb[:, s * H:e * H],
                    bd[N_ONETIME_BANDS + s:N_ONETIME_BANDS + e].rearrange(
                        "n p m -> p n m"))

            def band_st(i):
                return sb[:, i * H:(i + 1) * H].bitcast(MM)

            # persistent planes
            def plane(tag, dt=MM, memset=True):
                t = state.tile([H, FREE], dt, tag=tag)
                if memset:
                    nc.vector.memset(t[:].bitcast(f32), 0.0)
                return t

            h_pl = [plane("h0"), plane("h1")]
            c_pl = [plane("c0", f32), plane("c1", f32)]
            u_pl = plane("u")
            r_pl = [plane("r0"), plane("r1")]
            p1_pl = [plane("p1a", f32), plane("p1b", f32)]
            lp = state.tile([H, BL], f32, tag="lp", name="lp")
            nc.vector.memset(lp[:], 0.0)
            ones = state.tile([H, 1], f32, tag="ones", name="ones")
            nc.vector.memset(ones[:], 1.0)
            # bias columns: 0-7 bih, 8-9 bc1, 10-11 bc2, 12-13 bo1, 14 bci,
            # 15 = -bo2[1], 16 = final output bias
            cst = -16.0 * 128.0 * 128.0 * (float(bo2[1]) + HALF_LOG_2PI)
            bias_vals = (list(bih) + list(bc1) + list(bc2) + list(bo1)
                         + [float(bci[0]), -float(bo2[1]), cst])
            bias_t = state.tile([H, 17], f32, tag="bias", name="bias")
            for j, v in enumerate(bias_vals):
                nc.vector.memset(bias_t[:, j:j + 1], float(v))

            def bap(j, p=H):
                return bias_t[:p, j:j + 1]

            def new_plane(pool, src_dram, ci, tag, bufs=None):
                t = pool.tile([H, FREE], MM, tag=tag, name=tag, bufs=bufs)
                t3 = t[:, : BL * WB].rearrange("p (b w) -> p b w", w=WB)
                nc.vector.memset(t3[:, :, 0:1].bitcast(f32), 0.0)
                nc.vector.memset(t3[:, :, WB - 1:WB].bitcast(f32), 0.0)
                nc.vector.memset(t[:, BL * WB:].bitcast(f32), 0.0)
                nc.sync.dma_start(
                    t3[:, :, 1:129], src_dram[:, ci].rearrange("b h w -> h b w"))
                return t

            x_planes = {}

            def get_x(ci):
                if ci not in x_planes:
                    x_planes[ci] = new_plane(stream, xd, ci, "xpl")
                return x_planes[ci]

            # ---------------- cond phase ----------------
            with tc.tile_pool(name="otbands", bufs=2) as otp:
                GRP = 24
                ob_cur = [None]

                def load_group(g):
                    ob = otp.tile([H, GRP * H], MM, tag="ob", name="ob")
                    nc.sync.dma_start(
                        ob[:, :], bd[g * GRP:(g + 1) * GRP].rearrange(
                            "n p m -> p n m"))
                    ob_cur[0] = ob

                def band_ot(i):
                    j = i % GRP
                    return ob_cur[0][:, j * H:(j + 1) * H].bitcast(MM)

                # cond1: 16 -> 2, tanh
                pc = {}
                for co in range(2):
                    for k, (b0, n, c0, lo) in enumerate(CHUNKS):
                        pc[(co, k)] = psum.tile([H, 3 * WB], f32, tag="ps", name="ps")
                for ci in range(16):
                    if ci % 4 == 0:
                        load_group(ci // 4)
                    cpl = new_plane(stream, cd, ci, "cpl", bufs=2)
                    cf = cpl[:].bitcast(MM)
                    for co in range(2):
                        for k, (b0, n, c0, lo) in enumerate(CHUNKS):
                            for dx in (-1, 0, 1):
                                nc.tensor.matmul(
                                    pc[(co, k)][:, :n * WB],
                                    band_ot(ci * 6 + co * 3 + (dx + 1)),
                                    cf[:, c0 + dx:c0 + dx + n * WB],
                                    start=(ci == 0 and dx == -1),
                                    stop=(ci == 15 and dx == 1))
                tc_pl = [ctmp.tile([H, FREE], MM, tag="tc", name="tc") for _ in range(2)]
                for t in tc_pl:
                    nc.vector.memset(t[:].bitcast(f32), 0.0)
                for co in range(2):
                    for k, (b0, n, c0, lo) in enumerate(CHUNKS):
                        p3 = pc[(co, k)][:, :n * WB].rearrange(
                            "p (b w) -> p b w", w=WB)[:, :, lo:lo + 128]
                        nc.scalar.activation(
                            i3(tc_pl[co][:], b0, n, 1), p3, AF.Tanh,
                            bias=bap(8 + co))

                # cond2 -> cond_f planes; then partial1 = conv(cond_f)+bo1
                cf_pl = [ctmp.tile([H, FREE], MM, tag="cf", name="cf") for _ in range(2)]
                for t in cf_pl:
                    nc.vector.memset(t[:].bitcast(f32), 0.0)
                load_group(4)
                for dst, srcs, base, bias_col, out_pl in (
                        (cf_pl, tc_pl, 96, 10, None),
                        (None, cf_pl, 108, 12, p1_pl)):
                    tgt = dst if dst is not None else out_pl
                    for co in range(2):
                        for k, (b0, n, c0, lo) in enumerate(CHUNKS):
                            pq = psum.tile([H, 3 * WB], f32, tag="ps", name="ps")
                            first = True
                            for ci in range(2):
                                sf = srcs[ci][:].bitcast(MM)
                                for dx in (-1, 0, 1):
                                    nc.tensor.matmul(
                                        pq[:, :n * WB],
                                        band_ot(base + co * 6 + ci * 3 + dx + 1),
                                        sf[:, c0 + dx:c0 + dx + n * WB],
                                        start=first,
                                        stop=(ci == 1 and dx == 1))
                                    first = False
                            p3 = pq[:, :n * WB].rearrange(
                                "p (b w) -> p b w", w=WB)[:, :, lo:lo + 128]
                            nc.scalar.activation(
                                i3(tgt[co][:], b0, n, 1), p3, AF.Identity,
                                bias=bap(bias_col + co))

            # ---------------- steps ----------------
            def lp_tail(pq0, pq1, xt, b0, n, c0, lo):
                NN = n * WB
                e = tmp.tile([H, NN], f32, tag="tw", name="e")
                nc.scalar.activation(e[:], pq1[:, :NN], AF.Exp,
                                     bias=bap(15), scale=-1.0)
                d = tmp.tile([H, NN], f32, tag="tw", name="d")
                nc.vector.tensor_scalar(d[:], pq0[:, :NN], float(bo2[0]), None,
                                        OP.add)
                d2 = tmp.tile([H, NN], f32, tag="tw", name="d2")
                nc.vector.tensor_tensor(d2[:], xt[:, c0:c0 + NN].bitcast(f32), d[:],
                                        OP.subtract)
                z = tmp.tile([H, NN], f32, tag="tw", name="z")
                nc.vector.tensor_tensor(z[:], d2[:], e[:], OP.mult)
                s = tmp.tile([H, NN], f32, tag="tw", name="s")
                nc.scalar.activation(s[:], z[:], AF.Square,
                                     scale=0.7071067811865476)
                t = tmp.tile([H, NN], f32, tag="tw", name="t")
                nc.vector.tensor_tensor(t[:], s[:], pq1[:, :NN], OP.add)
                red = tmp.tile([H, n], f32, tag="tw", name="red")
                t3 = t[:].rearrange("p (b w) -> p b w", w=WB)[:, :, lo:lo + 128]
                nc.vector.reduce_sum(red[:], t3, AX.X)
                nc.vector.tensor_add(lp[:, b0:b0 + n], lp[:, b0:b0 + n], red[:])

            def head2_and_lp(xt_pl, b0, n, c0, lo):
                NN = n * WB
                pq = []
                for co in range(2):
                    q = psum.tile([H, 3 * WB], f32, tag="ps", name="ps")
                    first = True
                    for ci in range(2):
                        rf = r_pl[ci][:].bitcast(MM)
                        for dx in (-1, 0, 1):
                            nc.tensor.matmul(
                                q[:, :NN],
                                band_st(87 + co * 6 + ci * 3 + dx + 1),
                                rf[:, c0 + dx:c0 + dx + NN],
                                start=first, stop=(ci == 1 and dx == 1))
                            first = False
                    pq.append(q)
                lp_tail(pq[0], pq[1], xt_pl[:], b0, n, c0, lo)

            # step 0: feat = 0 -> r = relu(partial1)
            x0 = get_x(0)
            for (b0, n, c0, lo) in CHUNKS:
                for co in range(2):
                    nc.scalar.activation(
                        i3(r_pl[co][:], b0, n, 1),
                        i3(p1_pl[co][:], b0, n, 1), AF.Relu)
                head2_and_lp(x0, b0, n, c0, lo)

            for st in range(1, 16):
                xp = get_x(st - 1)
                xt = get_x(st)
                for (b0, n, c0, lo) in CHUNKS:
                    NN = n * WB
                    # u = conv(xp, Wci) + bci
                    pu = psum.tile([H, 3 * WB], f32, tag="ps", name="ps")
                    xf = xp[:].bitcast(MM)
                    for dx in (-1, 0, 1):
                        nc.tensor.matmul(pu[:, :NN], band_st(dx + 1),
                                         xf[:, c0 + dx:c0 + dx + NN],
                                         start=(dx == -1), stop=(dx == 1))
                    p3 = pu[:, :NN].rearrange(
                        "p (b w) -> p b w", w=WB)[:, :, lo:lo + 128]
                    nc.scalar.activation(i3(u_pl[:], b0, n, 1), p3,
                                         AF.Identity, bias=bap(14))
                    # gates
                    srcs = [h_pl[0], h_pl[1], u_pl]
                    pg = [None] * 8
                    for co in (0, 2, 4, 6, 1, 3, 5, 7):
                        g = psum.tile([H, 3 * WB], f32, tag="ps", name="ps")
                        first = True
                        for si, spl in enumerate(srcs):
                            sf = spl[:].bitcast(MM)
                            for dx in (-1, 0, 1):
                                nc.tensor.matmul(
                                    g[:, :NN],
                                    band_st(3 + co * 9 + si * 3 + dx + 1),
                                    sf[:, c0 + dx:c0 + dx + NN],
                                    start=first, stop=(si == 2 and dx == 1))
                                first = False
                        pg[co] = g
                    # LSTM pointwise (i,f,g,o = pg[0:2],[2:4],[4:6],[6:8])
                    for f in range(2):
                        ti = tmp.tile([H, NN], f32, tag="tw", name="ti")
                        nc.scalar.activation(ti[:], pg[f][:, :NN], AF.Sigmoid,
                                             bias=bap(f))
                        tg = tmp.tile([H, NN], f32, tag="tw", name="tg")
                        nc.scalar.activation(tg[:], pg[4 + f][:, :NN], AF.Tanh,
                                             bias=bap(4 + f))
                        tf = tmp.tile([H, NN], f32, tag="tw", name="tf")
                        nc.scalar.activation(tf[:], pg[2 + f][:, :NN],
                                             AF.Sigmoid, bias=bap(2 + f))
                        to = tmp.tile([H, NN], f32, tag="tw", name="to")
                        nc.scalar.activation(to[:], pg[6 + f][:, :NN],
                                             AF.Sigmoid, bias=bap(6 + f))
                        tig = tmp.tile([H, NN], f32, tag="tw", name="tig")
                        nc.vector.tensor_tensor(tig[:], ti[:], tg[:], OP.mult)
                        csl = c_pl[f][:, c0:c0 + NN]
                        nc.vector.tensor_tensor(csl, tf[:], csl, OP.mult)
                        nc.vector.tensor_tensor(csl, csl, tig[:], OP.add)
                        tc_ = tmp.tile([H, NN], f32, tag="tw", name="tc_")
                        nc.scalar.activation(tc_[:], csl, AF.Tanh)
                        to3 = to[:].rearrange(
                            "p (b w) -> p b w", w=WB)[:, :, lo:lo + 128]
                        tc3 = tc_[:].rearrange(
                            "p (b w) -> p b w", w=WB)[:, :, lo:lo + 128]
                        nc.vector.tensor_tensor(
                            i3(h_pl[f][:], b0, n, 1), to3, tc3, OP.mult)
                    # head1: r = relu(conv(h,Wo1[:, :, :2]) + partial1)
                    for co in range(2):
                        ph = psum.tile([H, 3 * WB], f32, tag="ps", name="ps")
                        first = True
                        for ci in range(2):
                            hf = h_pl[ci][:].bitcast(MM)
                            for dx in (-1, 0, 1):
                                nc.tensor.matmul(
                                    ph[:, :NN],
                                    band_st(75 + co * 6 + ci * 3 + dx + 1),
                                    hf[:, c0 + dx:c0 + dx + NN],
                                    start=first, stop=(ci == 1 and dx == 1))
                                first = False
                        hp = tmp.tile([H, NN], f32, tag="tw", name="hp")
                        nc.vector.tensor_tensor(
                            hp[:], ph[:, :NN], p1_pl[co][:, c0:c0 + NN], OP.add)
                        hp3 = hp[:].rearrange(
                            "p (b w) -> p b w", w=WB)[:, :, lo:lo + 128]
                        nc.scalar.activation(
                            i3(r_pl[co][:], b0, n, 1), hp3, AF.Relu)
                    head2_and_lp(xt, b0, n, c0, lo)

            # final: out = -(sum_p lp) - 16*128*128*(bo2[1] + HALF_LOG_2PI)
            po = psum.tile([BL, 1], f32, tag="ps", name="ps")
            nc.tensor.matmul(po[:], lp[:], ones[:], start=True, stop=True)
            osb = state.tile([BL, 1], f32, tag="osb", name="osb")
            nc.scalar.activation(osb[:], po[:], AF.Identity,
                                 scale=-1.0, bias=bap(16, BL))
            nc.sync.dma_start(od[:], osb[:])
    nc.compile()
    return nc


def kernel(**inputs):
    x = np.ascontiguousarray(inputs["x"], np.float32)
    cond = np.ascontiguousarray(inputs["cond"], np.float32)
    bands = _build_bands(
        np.asarray(inputs["Wci"], np.float32),
        np.asarray(inputs["Wc1"], np.float32),
        np.asarray(inputs["Wc2"], np.float32),
        np.asarray(inputs["Wo1"], np.float32),
        np.asarray(inputs["Wo2"], np.float32),
        np.asarray(inputs["Wih"], np.float32),
        np.asarray(inputs["Whh"], np.float32))
    nc = _build_program(
        np.asarray(inputs["bci"], np.float32),
        np.asarray(inputs["bc1"], np.float32),
        np.asarray(inputs["bc2"], np.float32),
        np.asarray(inputs["bo1"], np.float32),
        np.asarray(inputs["bo2"], np.float32),
        np.asarray(inputs["bih"], np.float32))
    from concourse.bass_utils import run_bass_kernel_spmd
    in_maps = [
        {"x": x[i * BL:(i + 1) * BL], "cond": cond[i * BL:(i + 1) * BL],
         "bands": bands}
        for i in range(NCORES)
    ]
    res = run_bass_kernel_spmd(nc, in_maps, list(range(NCORES)))
    out = np.concatenate(
        [res.results[i]["out"].reshape(BL) for i in range(NCORES)])
    return out.astype(np.float32)


if __name__ == "__main__":
    # smoke test with tiny random weights
    rng = np.random.default_rng(0)
    ins = {
        "x": rng.standard_normal((64, 16, 128, 128), np.float32),
        "cond": rng.standard_normal((64, 16, 128, 128), np.float32),
        "Wci": rng.standard_normal((3, 3, 1, 1), np.float32) * 0.1,
        "bci": np.zeros(1, np.float32),
        "Wc1": rng.standard_normal((3, 3, 16, 2), np.float32) * 0.1,
        "bc1": np.zeros(2, np.float32),
        "Wc2": rng.standard_normal((3, 3, 2, 2), np.float32) * 0.1,
        "bc2": np.zeros(2, np.float32),
        "Wo1": rng.standard_normal((3, 3, 4, 2), np.float32) * 0.1,
        "bo1": np.zeros(2, np.float32),
        "Wo2": rng.standard_normal((3, 3, 2, 2), np.float32) * 0.1,
        "bo2": np.zeros(2, np.float32),
        "Wih": rng.standard_normal((3, 3, 1, 8), np.float32) * 0.1,
        "bih": np.zeros(8, np.float32),
        "Whh": rng.standard_normal((3, 3, 2, 8), np.float32) * 0.1,
    }
    print(kernel(**ins)[:8])



# revision 4
# speedup vs baseline: 1.3391x; 1.3391x over previous
"""Trainium2 Bass kernel for AutoregressiveConvLSTM log-prob.

Strategy (v2)
-------------
Data-parallel over batch: 64 images -> 8 NeuronCores, 8 images each.

Layout: each plane is [H=128 partitions, FREE] where image b occupies
flat columns OFF+130*b .. OFF+130*b+129 (interior at +1..+128, one zero
pad column each side; OFF=2 leading zeros allow dx=-2 taps).

All 3x3 convs run on the TensorEngine as banded matmuls in fp8(e4m3)
with MatmulPerfMode.DoubleRow: each instruction computes
  psum += bandA.T @ movingA + bandB.T @ movingB
at 0.5 PE cycles per output column (4x the fp32r rate).  Band pairs are
host-built [128, 2, 128] fp8 tri/penta-diagonal matrices.  The dy taps
live in the band diagonals; dx taps are free-dim column offsets into
the zero pads.  Pair sources must share one SBUF tile, so the state
pack P = [128, 6, FREE] holds (h0, h1, cf0, cf1, r0, r1) and the x
stream holds (x, x-shifted-left-1) so taps pair across dx.  The
conv_in (1->1) conv is folded into Wih as a single 5x5 conv (exact for
bci=0; interior-exact otherwise), removing the u plane entirely.

Sigmoids are computed as 0.5*tanh(x/2)+0.5 (Act tanh + DVE
tensor_scalar) so every activation comes from one table set - no
LoadActFuncSet thrash.  Gate psums are [128, 2, 512] (co-pairs fused)
so one Act op covers both features.  LSTM pointwise math runs in bf16
on DVE (2x mode); h-writes (bf16*bf16->fp8) run on the idle Pool
engine.  Per-pixel log-prob terms reduce via tensor_tensor_reduce with
the lp column as both init and accumulator.
"""

import numpy as np
import ml_dtypes

B_FULL, C, H, W, F = 64, 16, 128, 128, 2
NCORES = 8
BL = B_FULL // NCORES            # images per core
WB = W + 2                       # per-image block width incl pads
OFF = 2                          # leading zero cols (dx=-2 reach)
FREE = OFF + BL * WB + 2
HALF_LOG_2PI = 0.9189385332046727
LN_SQRT2 = 0.34657359027997264

F8 = ml_dtypes.float8_e4m3
BF16 = ml_dtypes.bfloat16

# chunks: (b0, n_imgs); psum free cols = n*130
CHUNKS = [(0, 3), (3, 3), (6, 2)]


def _nz(v):
    return float(v) != 0.0


def _pair_layout(bci, bc1, bc2, bo1, bo2, bih):
    """Ordered (key -> (offset, count)) for the band-pair DRAM tensor.
    Depends only on which biases are nonzero, so the program builder can
    mirror it without the weights."""
    gb = [_nz(bih[g]) or _nz(bci[0]) for g in range(8)]
    L = []
    for co in range(2):
        L.append((f"c1_{co}", 24 + (1 if _nz(bc1[co]) else 0)))
    for co in range(2):
        L.append((f"c2_{co}", 3 + (1 if _nz(bc2[co]) else 0)))
    for g in range(8):
        L.append((f"g{g}", 6 + (1 if gb[g] else 0)))
    for co in range(2):
        L.append((f"h1_{co}", 6 + (1 if _nz(bo1[co]) else 0)))
    for co in range(2):
        L.append((f"h2_{co}", 3))
    off = {}
    o = 0
    for k, n in L:
        off[k] = (o, n)
        o += n
    return off, o


def _band3(w3):
    b = np.zeros((H, H), np.float32)
    for dy in (-1, 0, 1):
        ar = np.arange(max(0, -dy), H - max(0, dy))
        b[ar + dy, ar] = w3[dy + 1]
    return b


def _band5(w5):
    b = np.zeros((H, H), np.float32)
    for dy in (-2, -1, 0, 1, 2):
        ar = np.arange(max(0, -dy), H - max(0, dy))
        b[ar + dy, ar] = w5[dy + 2]
    return b


def _bias_band(v):
    b = np.zeros((H, H), np.float32)
    b[0, :] = v
    return b


_ZB = np.zeros((H, H), np.float32)


def _build_bands(Wci, Wc1, Wc2, Wo1, Wo2, Wih, Whh,
                 bci, bc1, bc2, bo1, bo2, bih):
    off, total = _pair_layout(bci, bc1, bc2, bo1, bo2, bih)
    bands = np.zeros((total, H, 2, H), np.float32)
    pos = {k: o for k, (o, n) in off.items()}

    def emit(key, a, b):
        i = pos[key]
        bands[i, :, 0, :] = a
        bands[i, :, 1, :] = b
        pos[key] = i + 1

    # cond1: 16 -> 2; channel pairs (2k, 2k+1)
    for co in range(2):
        k0 = f"c1_{co}"
        for k in range(8):
            for dx in range(3):
                emit(k0, _band3(Wc1[:, dx, 2 * k, co]),
                     _band3(Wc1[:, dx, 2 * k + 1, co]))
        if _nz(bc1[co]):
            emit(k0, _bias_band(bc1[co]), _ZB)
    # cond2: 2 -> 2
    for co in range(2):
        k0 = f"c2_{co}"
        for dx in range(3):
            emit(k0, _band3(Wc2[:, dx, 0, co]), _band3(Wc2[:, dx, 1, co]))
        if _nz(bc2[co]):
            emit(k0, _bias_band(bc2[co]), _ZB)
    # gates: 5x5 composite of Wci then Wih, plus Whh
    W5 = np.zeros((5, 5, 8), np.float32)
    for co in range(8):
        for a in range(3):
            for d in range(3):
                for b in range(3):
                    for e in range(3):
                        W5[a + b, d + e, co] += (
                            Wci[a, d, 0, 0] * Wih[b, e, 0, co])
    gbias = [float(bih[co]) + float(bci[0]) * float(Wih[:, :, 0, co].sum())
             for co in range(8)]
    for co in range(8):
        k0 = f"g{co}"
        for dx in range(3):
            emit(k0, _band3(Whh[:, dx, 0, co]), _band3(Whh[:, dx, 1, co]))
        emit(k0, _band5(W5[:, 0, co]), _band5(W5[:, 1, co]))   # xbase -2
        emit(k0, _band5(W5[:, 2, co]), _band5(W5[:, 3, co]))   # xbase 0
        emit(k0, _band5(W5[:, 4, co]), _ZB)                     # xbase +2
        if _nz(bih[co]) or _nz(bci[0]):
            emit(k0, _bias_band(gbias[co]), _ZB)
    # head1: feat part + cond part of Wo1
    for co in range(2):
        k0 = f"h1_{co}"
        for dx in range(3):
            emit(k0, _band3(Wo1[:, dx, 0, co]), _band3(Wo1[:, dx, 1, co]))
        for dx in range(3):
            emit(k0, _band3(Wo1[:, dx, 2, co]), _band3(Wo1[:, dx, 3, co]))
        if _nz(bo1[co]):
            emit(k0, _bias_band(bo1[co]), _ZB)
    # head2
    for co in range(2):
        k0 = f"h2_{co}"
        for dx in range(3):
            emit(k0, _band3(Wo2[:, dx, 0, co]), _band3(Wo2[:, dx, 1, co]))
    for k, (o, n) in off.items():
        assert pos[k] == o + n, (k, pos[k], o, n)
    return bands.astype(F8), off, total


def _build_program(bci, bc1, bc2, bo1, bo2, bih):
    import concourse.bacc as bacc
    import concourse.mybir as mybir
    import concourse.tile as tile

    f32 = mybir.dt.float32
    f8 = mybir.dt.float8e4
    bf = mybir.dt.bfloat16
    AF = mybir.ActivationFunctionType
    OP = mybir.AluOpType
    DR = mybir.MatmulPerfMode.DoubleRow

    off, NP = _pair_layout(bci, bc1, bc2, bo1, bo2, bih)
    n_ot = off["g0"][0]                      # one-time pairs (cond)
    n_res = NP - n_ot                        # resident pairs

    nc = bacc.Bacc("TRN2", target_bir_lowering=False, debug=False)
    xd8 = nc.dram_tensor("x8", [C - 1, 2, H, FREE], f8, kind="ExternalInput")
    xbd = nc.dram_tensor("xb", [C, H, FREE], bf, kind="ExternalInput")
    cdd = nc.dram_tensor("c8", [8, 2, H, FREE], f8, kind="ExternalInput")
    bdd = nc.dram_tensor("bands", [NP, H, 2, H], f8, kind="ExternalInput")
    od = nc.dram_tensor("out", [BL, 1], f32, kind="ExternalOutput")

    def BS(b):
        return OFF + b * WB

    with tile.TileContext(nc) as tc:
        import contextlib
        ctx = contextlib.ExitStack()
        with ctx:
            state = ctx.enter_context(tc.tile_pool(name="state", bufs=1))
            sbands = ctx.enter_context(tc.tile_pool(name="sbands", bufs=1))
            xstream = ctx.enter_context(tc.tile_pool(name="xs", bufs=3))
            bstream = ctx.enter_context(tc.tile_pool(name="bs", bufs=3))
            tmp = ctx.enter_context(tc.tile_pool(name="tmp", bufs=24))
            psum = ctx.enter_context(
                tc.tile_pool(name="psum", bufs=4, space="PSUM"))

            # resident band pairs
            sb = sbands.tile([H, n_res, 2, H], f8, tag="sb", name="sb")
            for k in range(8):
                s = (n_res * k) // 8
                e = (n_res * (k + 1)) // 8
                nc.sync.dma_start(
                    sb[:, s:e],
                    bdd[n_ot + s:n_ot + e].rearrange("n p t m -> p n t m"))

            def bp(key, j):
                o, n = off[key]
                assert j < n
                return sb[:, o - n_ot + j]

            # persistent state
            P = state.tile([H, 6, FREE], f8, tag="P", name="P")
            nc.gpsimd.memset(P[:], 0.0)
            cst_t = state.tile([H, 2, FREE], bf, tag="c", name="c")
            nc.vector.memset(cst_t[:], 0.0)
            ones8 = state.tile([H, 2, WB + 2], f8, tag="o8", name="o8")
            nc.vector.memset(ones8[:], 1.0)
            lp = state.tile([H, BL], f32, tag="lp", name="lp")
            nc.vector.memset(lp[:], 0.0)
            ones_f = state.tile([H, 1], f32, tag="of", name="of")
            nc.vector.memset(ones_f[:], 1.0)
            # bias cols: 0 = exp bias, 1 = final output bias
            cstv = -16.0 * 128.0 * 128.0 * (float(bo2[1]) + HALF_LOG_2PI)
            bias_t = state.tile([H, 2], f32, tag="bias", name="bias")
            nc.vector.memset(bias_t[:, 0:1], -float(bo2[1]) - LN_SQRT2)
            nc.vector.memset(bias_t[:, 1:2], cstv)

            def interior(ap_flat):
                # [p, s, NN] -> [p, s, n, 128]
                return ap_flat.rearrange("p s (b w) -> p s b w", w=WB)[
                    :, :, :, 1:129]

            def head(key_pfx, src_slots, Tg, b0, n, include_h=True,
                     bias_flags=(False, False)):
                # head1/head2-style group: co at dim1 of Tg
                for co in range(2):
                    key = f"{key_pfx}_{co}"
                    for j in range(n):
                        base = BS(b0 + j)
                        out = Tg[:, co, j * 130:(j + 1) * 130]
                        idx = 0
                        first = True
                        tot = off[key][1]
                        skip = 3 if (key_pfx == "h1" and not include_h) else 0
                        for dx in (-1, 0, 1):
                            if skip:
                                idx += 1
                                continue
                            nc.tensor.matmul(
                                out, bp(key, idx),
                                P[:, src_slots[0]:src_slots[0] + 2,
                                  base + dx:base + dx + 130],
                                start=first, stop=(idx == tot - 1),
                                perf_mode=DR)
                            first = False
                            idx += 1
                        if key_pfx == "h1":
                            for dx in (-1, 0, 1):
                                nc.tensor.matmul(
                                    out, bp(key, idx),
                                    P[:, 2:4, base + dx:base + dx + 130],
                                    start=first, stop=(idx == tot - 1),
                                    perf_mode=DR)
                                first = False
                                idx += 1
                        if bias_flags[co]:
                            nc.tensor.matmul(
                                out, bp(key, idx), ones8[:, :, 0:130],
                                start=first, stop=True, perf_mode=DR)
                            idx += 1

            def head2_and_lp(b0, n, xbt):
                NN = n * 130
                Th2 = psum.tile([H, 2, 512], f32, tag="ps", name="ps")
                head("h2", (4, 4), Th2, b0, n)
                e = tmp.tile([H, NN], bf, tag="tw", name="e")
                nc.scalar.activation(e[:], Th2[:, 1, :NN], AF.Exp,
                                     bias=bias_t[:, 0:1], scale=-1.0)
                d2 = tmp.tile([H, NN], bf, tag="tw", name="d2")
                nc.vector.tensor_tensor(
                    d2[:], xbt[:, BS(b0):BS(b0) + NN], Th2[:, 0, :NN],
                    OP.subtract)
                z = tmp.tile([H, NN], bf, tag="tw", name="z")
                nc.vector.tensor_tensor(z[:], d2[:], e[:], OP.mult)
                z2 = tmp.tile([H, NN], bf, tag="tw", name="z2")
                nc.vector.tensor_tensor(z2[:], z[:], z[:], OP.mult)
                for j in range(n):
                    b = b0 + j
                    scrap = tmp.tile([H, 128], bf, tag="tw", name="scr")
                    nc.vector.tensor_tensor_reduce(
                        scrap[:],
                        z2[:, j * 130 + 1:j * 130 + 129],
                        Th2[:, 1, j * 130 + 1:j * 130 + 129],
                        1.0, lp[:, b:b + 1], OP.add, OP.add,
                        lp[:, b:b + 1])

            # ---------------- cond phase ----------------
            with tc.tile_pool(name="otb", bufs=1) as otp, \
                 tc.tile_pool(name="cstr", bufs=2) as cstr:
                ot = otp.tile([H, n_ot, 2, H], f8, tag="ot", name="ot")
                for k in range(4):
                    s = (n_ot * k) // 4
                    e = (n_ot * (k + 1)) // 4
                    nc.sync.dma_start(
                        ot[:, s:e],
                        bdd[s:e].rearrange("n p t m -> p n t m"))

                def bot(key, j):
                    o, n = off[key]
                    assert j < n
                    return ot[:, o + j]

                tc8 = state.tile([H, 2, FREE], f8, tag="tc8", name="tc8")
                nc.vector.memset(tc8[:], 0.0)

                pc = {}
                for ci, (b0, n) in enumerate(CHUNKS):
                    pc[ci] = psum.tile([H, 2, 512], f32, tag="ps", name="ps")
                for k in range(8):
                    cpl = cstr.tile([H, 2, FREE], f8, tag="cpl", name="cpl")
                    nc.sync.dma_start(
                        cpl[:], cdd[k].rearrange("t h w -> h t w"))
                    for ci, (b0, n) in enumerate(CHUNKS):
                        for co in range(2):
                            key = f"c1_{co}"
                            tot = off[key][1]
                            for j in range(n):
                                base = BS(b0 + j)
                                out = pc[ci][:, co, j * 130:(j + 1) * 130]
                                for dxi, dx in enumerate((-1, 0, 1)):
                                    idx = k * 3 + dxi
                                    last = (idx == tot - 1)
                                    nc.tensor.matmul(
                                        out, bot(key, idx),
                                        cpl[:, :, base + dx:base + dx + 130],
                                        start=(idx == 0), stop=last,
                                        perf_mode=DR)
                                if k == 7 and _nz(bc1[co]):
                                    nc.tensor.matmul(
                                        out, bot(key, tot - 1),
                                        ones8[:, :, 0:130],
                                        start=False, stop=True, perf_mode=DR)
                for ci, (b0, n) in enumerate(CHUNKS):
                    NN = n * 130
                    nc.scalar.activation(
                        interior(tc8[:, :, BS(b0):BS(b0) + NN]),
                        interior(pc[ci][:, :, :NN]), AF.Tanh)
                # cond2 -> cf slots of P
                for ci, (b0, n) in enumerate(CHUNKS):
                    NN = n * 130
                    pq = psum.tile([H, 2, 512], f32, tag="ps", name="ps")
                    for co in range(2):
                        key = f"c2_{co}"
                        tot = off[key][1]
                        for j in range(n):
                            base = BS(b0 + j)
                            out = pq[:, co, j * 130:(j + 1) * 130]
                            for dxi, dx in enumerate((-1, 0, 1)):
                                nc.tensor.matmul(
                                    out, bot(key, dxi),
                                    tc8[:, :, base + dx:base + dx + 130],
                                    start=(dxi == 0), stop=(dxi == tot - 1),
                                    perf_mode=DR)
                            if _nz(bc2[co]):
                                nc.tensor.matmul(
                                    out, bot(key, tot - 1),
                                    ones8[:, :, 0:130],
                                    start=False, stop=True, perf_mode=DR)
                    nc.scalar.activation(
                        interior(P[:, 2:4, BS(b0):BS(b0) + NN]),
                        interior(pq[:, :, :NN]), AF.Identity)

            # ---------------- step 0 ----------------
            xbt0 = bstream.tile([H, FREE], bf, tag="xbt", name="xbt")
            nc.sync.dma_start(xbt0[:], xbd[0])
            h1b = (_nz(bo1[0]), _nz(bo1[1]))
            for (b0, n) in CHUNKS:
                NN = n * 130
                Th1 = psum.tile([H, 2, 512], f32, tag="ps", name="ps")
                head("h1", (0, 0), Th1, b0, n, include_h=False,
                     bias_flags=h1b)
                nc.vector.tensor_scalar(
                    interior(P[:, 4:6, BS(b0):BS(b0) + NN]),
                    interior(Th1[:, :, :NN]), 0.0, None, OP.max)
                head2_and_lp(b0, n, xbt0)

            # ---------------- steps ----------------
            gb = [_nz(bih[g]) or _nz(bci[0]) for g in range(8)]
            for st in range(1, 16):
                xpl = xstream.tile([H, 2, FREE], f8, tag="xpl", name="xpl")
                nc.sync.dma_start(xpl[:], xd8[st - 1].rearrange(
                    "t h w -> h t w"))
                xbt = bstream.tile([H, FREE], bf, tag="xbt", name="xbt")
                nc.sync.dma_start(xbt[:], xbd[st])
                for (b0, n) in CHUNKS:
                    NN = n * 130
                    tnh = []
                    for g in range(4):
                        Tg = psum.tile([H, 2, 512], f32, tag="ps", name="ps")
                        for f01 in range(2):
                            co = 2 * g + f01
                            key = f"g{co}"
                            tot = off[key][1]
                            for j in range(n):
                                base = BS(b0 + j)
                                out = Tg[:, f01, j * 130:(j + 1) * 130]
                                idx = 0
                                for dx in (-1, 0, 1):
                                    nc.tensor.matmul(
                                        out, bp(key, idx),
                                        P[:, 0:2, base + dx:base + dx + 130],
                                        start=(idx == 0),
                                        stop=(idx == tot - 1), perf_mode=DR)
                                    idx += 1
                                for xb_ in (-2, 0, 2):
                                    nc.tensor.matmul(
                                        out, bp(key, idx),
                                        xpl[:, :, base + xb_:base + xb_ + 130],
                                        start=False, stop=(idx == tot - 1),
                                        perf_mode=DR)
                                    idx += 1
                                if gb[co]:
                                    nc.tensor.matmul(
                                        out, bp(key, idx),
                                        ones8[:, :, 0:130],
                                        start=False, stop=True, perf_mode=DR)
                        t = tmp.tile([H, 2, NN], bf, tag="tw", name="tnh")
                        nc.scalar.activation(
                            t[:], Tg[:, :, :NN], AF.Tanh,
                            scale=(1.0 if g == 2 else 0.5))
                        tnh.append(t)
                    # sigmoids via 0.5*tanh+0.5
                    sig = {}
                    for g in (0, 1, 3):
                        s = tmp.tile([H, 2, NN], bf, tag="tw", name="sig")
                        nc.vector.tensor_scalar(
                            s[:], tnh[g][:], 0.5, 0.5, OP.mult, OP.add)
                        sig[g] = s
                    tig = tmp.tile([H, 2, NN], bf, tag="tw", name="tig")
                    nc.vector.tensor_tensor(
                        tig[:], sig[0][:], tnh[2][:], OP.mult)
                    cs = cst_t[:, :, BS(b0):BS(b0) + NN]
                    nc.vector.tensor_tensor(cs, sig[1][:], cs, OP.mult)
                    nc.vector.tensor_tensor(cs, cs, tig[:], OP.add)
                    tcc = tmp.tile([H, 2, NN], bf, tag="tw", name="tcc")
                    nc.scalar.activation(tcc[:], cs, AF.Tanh)
                    # h write (Pool): fp8 pack slots 0,1
                    nc.gpsimd.tensor_tensor(
                        interior(P[:, 0:2, BS(b0):BS(b0) + NN]),
                        interior(sig[3][:]), interior(tcc[:]), OP.mult)
                    # head1 -> relu -> r slots
                    Th1 = psum.tile([H, 2, 512], f32, tag="ps", name="ps")
                    head("h1", (0, 0), Th1, b0, n, bias_flags=h1b)
                    nc.vector.tensor_scalar(
                        interior(P[:, 4:6, BS(b0):BS(b0) + NN]),
                        interior(Th1[:, :, :NN]), 0.0, None, OP.max)
                    head2_and_lp(b0, n, xbt)

            # final (reuse a main psum-pool tile for the [8,1] reduction)
            po_t = psum.tile([H, 2, 512], f32, tag="ps", name="ps")
            po = po_t[:BL, 0, 0:1]
            nc.tensor.matmul(po, lp[:], ones_f[:], start=True, stop=True)
            osb = state.tile([BL, 1], f32, tag="osb", name="osb")
            nc.scalar.activation(osb[:], po, AF.Identity,
                                 scale=-1.0, bias=bias_t[:BL, 1:2])
            nc.sync.dma_start(od[:], osb[:])
    nc.compile()
    return nc


def _host_inputs(inputs):
    x = np.ascontiguousarray(inputs["x"], np.float32)
    cond = np.ascontiguousarray(inputs["cond"], np.float32)
    bo2 = np.asarray(inputs["bo2"], np.float32)

    bands, off, NP = _build_bands(
        np.asarray(inputs["Wci"], np.float32),
        np.asarray(inputs["Wc1"], np.float32),
        np.asarray(inputs["Wc2"], np.float32),
        np.asarray(inputs["Wo1"], np.float32),
        np.asarray(inputs["Wo2"], np.float32),
        np.asarray(inputs["Wih"], np.float32),
        np.asarray(inputs["Whh"], np.float32),
        np.asarray(inputs["bci"], np.float32),
        np.asarray(inputs["bc1"], np.float32),
        np.asarray(inputs["bc2"], np.float32),
        np.asarray(inputs["bo1"], np.float32),
        bo2,
        np.asarray(inputs["bih"], np.float32))

    x8 = np.zeros((NCORES, C - 1, 2, H, FREE), F8)
    xb = np.zeros((NCORES, C, H, FREE), BF16)
    c8 = np.zeros((NCORES, 8, 2, H, FREE), F8)
    for core in range(NCORES):
        xs = x[core * BL:(core + 1) * BL]        # [8, C, H, W]
        cs = cond[core * BL:(core + 1) * BL]
        A = np.zeros((C, H, FREE), np.float32)
        Ac = np.zeros((C, H, FREE), np.float32)
        for b in range(BL):
            s = OFF + b * WB + 1
            A[:, :, s:s + 128] = xs[b]
            Ac[:, :, s:s + 128] = cs[b]
        A8 = A.astype(F8)
        x8[core, :, 0] = A8[:C - 1]
        x8[core, :C - 1, 1, :, :-1] = A8[:C - 1, :, 1:]
        xb[core] = (A - float(bo2[0])).astype(BF16)
        Ac8 = Ac.astype(F8)
        c8[core, :, 0] = Ac8[0::2]
        c8[core, :, 1] = Ac8[1::2]
    return x8, xb, c8, bands


def kernel(**inputs):
    x8, xb, c8, bands = _host_inputs(inputs)
    nc = _build_program(
        np.asarray(inputs["bci"], np.float32),
        np.asarray(inputs["bc1"], np.float32),
        np.asarray(inputs["bc2"], np.float32),
        np.asarray(inputs["bo1"], np.float32),
        np.asarray(inputs["bo2"], np.float32),
        np.asarray(inputs["bih"], np.float32))
    from concourse.bass_utils import run_bass_kernel_spmd
    in_maps = [
        {"x8": x8[i], "xb": xb[i], "c8": c8[i], "bands": bands}
        for i in range(NCORES)
    ]
    res = run_bass_kernel_spmd(nc, in_maps, list(range(NCORES)))
    out = np.concatenate(
        [res.results[i]["out"].reshape(BL) for i in range(NCORES)])
    return out.astype(np.float32)


if __name__ == "__main__":
    rng = np.random.default_rng(0)
    ins = {
        "x": rng.standard_normal((64, 16, 128, 128)).astype(np.float32),
        "cond": rng.standard_normal((64, 16, 128, 128)).astype(np.float32),
        "Wci": (rng.standard_normal((3, 3, 1, 1)) * 0.1).astype(np.float32),
        "bci": np.zeros(1, np.float32),
        "Wc1": (rng.standard_normal((3, 3, 16, 2)) * 0.1).astype(np.float32),
        "bc1": np.zeros(2, np.float32),
        "Wc2": (rng.standard_normal((3, 3, 2, 2)) * 0.1).astype(np.float32),
        "bc2": np.zeros(2, np.float32),
        "Wo1": (rng.standard_normal((3, 3, 4, 2)) * 0.1).astype(np.float32),
        "bo1": np.zeros(2, np.float32),
        "Wo2": (rng.standard_normal((3, 3, 2, 2)) * 0.1).astype(np.float32),
        "bo2": np.zeros(2, np.float32),
        "Wih": (rng.standard_normal((3, 3, 1, 8)) * 0.1).astype(np.float32),
        "bih": np.zeros(8, np.float32),
        "Whh": (rng.standard_normal((3, 3, 2, 8)) * 0.1).astype(np.float32),
    }
    print(kernel(**ins)[:8])


# revision 6
# speedup vs baseline: 1.5959x; 1.1918x over previous
"""Trainium2 Bass kernel for AutoregressiveConvLSTM log-prob.

Strategy (v2)
-------------
Data-parallel over batch: 64 images -> 8 NeuronCores, 8 images each.

Layout: each plane is [H=128 partitions, FREE] where image b occupies
flat columns OFF+130*b .. OFF+130*b+129 (interior at +1..+128, one zero
pad column each side; OFF=2 leading zeros allow dx=-2 taps).

All 3x3 convs run on the TensorEngine as banded matmuls in fp8(e4m3)
with MatmulPerfMode.DoubleRow: each instruction computes
  psum += bandA.T @ movingA + bandB.T @ movingB
at 0.5 PE cycles per output column (4x the fp32r rate).  Band pairs are
host-built [128, 2, 128] fp8 tri/penta-diagonal matrices.  The dy taps
live in the band diagonals; dx taps are free-dim column offsets into
the zero pads.  Pair sources must share one SBUF tile, so the state
pack P = [128, 6, FREE] holds (h0, h1, cf0, cf1, r0, r1) and the x
stream holds (x, x-shifted-left-1) so taps pair across dx.  The
conv_in (1->1) conv is folded into Wih as a single 5x5 conv (exact for
bci=0; interior-exact otherwise), removing the u plane entirely.

Sigmoids are computed as 0.5*tanh(x/2)+0.5 (Act tanh + DVE
tensor_scalar) so every activation comes from one table set - no
LoadActFuncSet thrash.  Gate psums are [128, 2, 512] (co-pairs fused)
so one Act op covers both features.  LSTM pointwise math runs in bf16
on DVE (2x mode); h-writes (bf16*bf16->fp8) run on the idle Pool
engine.  Per-pixel log-prob terms reduce via tensor_tensor_reduce with
the lp column as both init and accumulator.
"""

import numpy as np
import ml_dtypes

B_FULL, C, H, W, F = 64, 16, 128, 128, 2
NCORES = 8
BL = B_FULL // NCORES            # images per core
WB = W + 2                       # per-image block width incl pads
OFF = 2                          # leading zero cols (dx=-2 reach)
FREE = OFF + BL * WB + 2
HALF_LOG_2PI = 0.9189385332046727
LN_SQRT2 = 0.34657359027997264

F8 = ml_dtypes.float8_e4m3
BF16 = ml_dtypes.bfloat16

# chunks: (b0, n_imgs); psum free cols = n*130
CHUNKS = [(0, 3), (3, 3), (6, 2)]


def _nz(v):
    return float(v) != 0.0


def _pair_layout(bci, bc1, bc2, bo1, bo2, bih):
    """Ordered (key -> (offset, count)) for the band-pair DRAM tensor.
    Depends only on which biases are nonzero, so the program builder can
    mirror it without the weights."""
    gb = [_nz(bih[g]) or _nz(bci[0]) for g in range(8)]
    L = []
    for co in range(2):
        L.append((f"c1_{co}", 24 + (1 if _nz(bc1[co]) else 0)))
    for co in range(2):
        L.append((f"c2_{co}", 3 + (1 if _nz(bc2[co]) else 0)))
    for g in range(8):
        L.append((f"g{g}", 6 + (1 if gb[g] else 0)))
    for co in range(2):
        L.append((f"h1_{co}", 6 + (1 if _nz(bo1[co]) else 0)))
    for co in range(2):
        L.append((f"h2_{co}", 3))
    off = {}
    o = 0
    for k, n in L:
        off[k] = (o, n)
        o += n
    return off, o


def _band3(w3):
    b = np.zeros((H, H), np.float32)
    for dy in (-1, 0, 1):
        ar = np.arange(max(0, -dy), H - max(0, dy))
        b[ar + dy, ar] = w3[dy + 1]
    return b


def _band5(w5):
    b = np.zeros((H, H), np.float32)
    for dy in (-2, -1, 0, 1, 2):
        ar = np.arange(max(0, -dy), H - max(0, dy))
        b[ar + dy, ar] = w5[dy + 2]
    return b


def _bias_band(v):
    b = np.zeros((H, H), np.float32)
    b[0, :] = v
    return b


_ZB = np.zeros((H, H), np.float32)


def _build_bands(Wci, Wc1, Wc2, Wo1, Wo2, Wih, Whh,
                 bci, bc1, bc2, bo1, bo2, bih):
    off, total = _pair_layout(bci, bc1, bc2, bo1, bo2, bih)
    bands = np.zeros((total, H, 2, H), np.float32)
    pos = {k: o for k, (o, n) in off.items()}

    def emit(key, a, b):
        i = pos[key]
        bands[i, :, 0, :] = a
        bands[i, :, 1, :] = b
        pos[key] = i + 1

    # cond1: 16 -> 2; channel pairs (2k, 2k+1)
    for co in range(2):
        k0 = f"c1_{co}"
        for k in range(8):
            for dx in range(3):
                emit(k0, _band3(Wc1[:, dx, 2 * k, co]),
                     _band3(Wc1[:, dx, 2 * k + 1, co]))
        if _nz(bc1[co]):
            emit(k0, _bias_band(bc1[co]), _ZB)
    # cond2: 2 -> 2
    for co in range(2):
        k0 = f"c2_{co}"
        for dx in range(3):
            emit(k0, _band3(Wc2[:, dx, 0, co]), _band3(Wc2[:, dx, 1, co]))
        if _nz(bc2[co]):
            emit(k0, _bias_band(bc2[co]), _ZB)
    # gates: 5x5 composite of Wci then Wih, plus Whh
    W5 = np.zeros((5, 5, 8), np.float32)
    for co in range(8):
        for a in range(3):
            for d in range(3):
                for b in range(3):
                    for e in range(3):
                        W5[a + b, d + e, co] += (
                            Wci[a, d, 0, 0] * Wih[b, e, 0, co])
    gbias = [float(bih[co]) + float(bci[0]) * float(Wih[:, :, 0, co].sum())
             for co in range(8)]
    for co in range(8):
        k0 = f"g{co}"
        for dx in range(3):
            emit(k0, _band3(Whh[:, dx, 0, co]), _band3(Whh[:, dx, 1, co]))
        emit(k0, _band5(W5[:, 0, co]), _band5(W5[:, 1, co]))   # xbase -2
        emit(k0, _band5(W5[:, 2, co]), _band5(W5[:, 3, co]))   # xbase 0
        emit(k0, _band5(W5[:, 4, co]), _ZB)                     # xbase +2
        if _nz(bih[co]) or _nz(bci[0]):
            emit(k0, _bias_band(gbias[co]), _ZB)
    # head1: feat part + cond part of Wo1
    for co in range(2):
        k0 = f"h1_{co}"
        for dx in range(3):
            emit(k0, _band3(Wo1[:, dx, 0, co]), _band3(Wo1[:, dx, 1, co]))
        for dx in range(3):
            emit(k0, _band3(Wo1[:, dx, 2, co]), _band3(Wo1[:, dx, 3, co]))
        if _nz(bo1[co]):
            emit(k0, _bias_band(bo1[co]), _ZB)
    # head2
    for co in range(2):
        k0 = f"h2_{co}"
        for dx in range(3):
            emit(k0, _band3(Wo2[:, dx, 0, co]), _band3(Wo2[:, dx, 1, co]))
    for k, (o, n) in off.items():
        assert pos[k] == o + n, (k, pos[k], o, n)
    return bands.astype(F8), off, total


def _build_program(bci, bc1, bc2, bo1, bo2, bih):
    import concourse.bacc as bacc
    import concourse.mybir as mybir
    import concourse.tile as tile

    f32 = mybir.dt.float32
    f8 = mybir.dt.float8e4
    bf = mybir.dt.bfloat16
    AF = mybir.ActivationFunctionType
    OP = mybir.AluOpType
    DR = mybir.MatmulPerfMode.DoubleRow

    off, NP = _pair_layout(bci, bc1, bc2, bo1, bo2, bih)
    n_ot = off["g0"][0]                      # one-time pairs (cond)
    n_res = NP - n_ot                        # resident pairs

    nc = bacc.Bacc("TRN2", target_bir_lowering=False, debug=False)
    xd8 = nc.dram_tensor("x8", [C - 1, 2, H, FREE], f8, kind="ExternalInput")
    xbd = nc.dram_tensor("xb", [C, H, FREE], bf, kind="ExternalInput")
    cdd = nc.dram_tensor("c8", [8, 2, H, FREE], f8, kind="ExternalInput")
    bdd = nc.dram_tensor("bands", [NP, H, 2, H], f8, kind="ExternalInput")
    od = nc.dram_tensor("out", [BL, 1], f32, kind="ExternalOutput")

    def BS(b):
        return OFF + b * WB

    with tile.TileContext(nc) as tc:
        import contextlib
        ctx = contextlib.ExitStack()
        with ctx:
            state = ctx.enter_context(tc.tile_pool(name="state", bufs=1))
            sbands = ctx.enter_context(tc.tile_pool(name="sbands", bufs=1))
            xstream = ctx.enter_context(tc.tile_pool(name="xs", bufs=3))
            bstream = ctx.enter_context(tc.tile_pool(name="bs", bufs=3))
            tmp = ctx.enter_context(tc.tile_pool(name="tmp", bufs=24))
            psum = ctx.enter_context(
                tc.tile_pool(name="psum", bufs=4, space="PSUM"))

            # resident band pairs
            sb = sbands.tile([H, n_res, 2, H], f8, tag="sb", name="sb")
            for k in range(8):
                s = (n_res * k) // 8
                e = (n_res * (k + 1)) // 8
                nc.sync.dma_start(
                    sb[:, s:e],
                    bdd[n_ot + s:n_ot + e].rearrange("n p t m -> p n t m"))

            def bp(key, j):
                o, n = off[key]
                assert j < n
                return sb[:, o - n_ot + j]

            # persistent state
            P = state.tile([H, 6, FREE], f8, tag="P", name="P")
            nc.gpsimd.memset(P[:], 0.0)
            cst_t = state.tile([H, 2, FREE], bf, tag="c", name="c")
            nc.vector.memset(cst_t[:], 0.0)
            ones8 = state.tile([H, 2, WB + 2], f8, tag="o8", name="o8")
            nc.vector.memset(ones8[:], 1.0)
            lp = state.tile([H, BL], f32, tag="lp", name="lp")
            nc.vector.memset(lp[:], 0.0)
            ones_f = state.tile([H, 1], f32, tag="of", name="of")
            nc.vector.memset(ones_f[:], 1.0)
            # bias cols: 0 = exp bias, 1 = final output bias
            cstv = -16.0 * 128.0 * 128.0 * (float(bo2[1]) + HALF_LOG_2PI)
            bias_t = state.tile([H, 2], f32, tag="bias", name="bias")
            nc.vector.memset(bias_t[:, 0:1], -float(bo2[1]) - LN_SQRT2)
            nc.vector.memset(bias_t[:, 1:2], cstv)

            def interior(ap_flat):
                # [p, s, NN] -> [p, s, n, 128]
                return ap_flat.rearrange("p s (b w) -> p s b w", w=WB)[
                    :, :, :, 1:129]

            def head(key_pfx, src_slots, Tg, b0, n, include_h=True,
                     bias_flags=(False, False)):
                # head1/head2-style group: co at dim1 of Tg
                for co in range(2):
                    key = f"{key_pfx}_{co}"
                    for j in range(n):
                        base = BS(b0 + j)
                        out = Tg[:, co, j * 130:(j + 1) * 130]
                        idx = 0
                        first = True
                        tot = off[key][1]
                        skip = 3 if (key_pfx == "h1" and not include_h) else 0
                        for dx in (-1, 0, 1):
                            if skip:
                                idx += 1
                                continue
                            nc.tensor.matmul(
                                out, bp(key, idx),
                                P[:, src_slots[0]:src_slots[0] + 2,
                                  base + dx:base + dx + 130],
                                start=first, stop=(idx == tot - 1),
                                perf_mode=DR)
                            first = False
                            idx += 1
                        if key_pfx == "h1":
                            for dx in (-1, 0, 1):
                                nc.tensor.matmul(
                                    out, bp(key, idx),
                                    P[:, 2:4, base + dx:base + dx + 130],
                                    start=first, stop=(idx == tot - 1),
                                    perf_mode=DR)
                                first = False
                                idx += 1
                        if bias_flags[co]:
                            nc.tensor.matmul(
                                out, bp(key, idx), ones8[:, :, 0:130],
                                start=first, stop=True, perf_mode=DR)
                            idx += 1

            def head2_and_lp(b0, n, xbt):
                NN = n * 130
                Th2 = psum.tile([H, 2, 512], f32, tag="ps", name="ps")
                head("h2", (4, 4), Th2, b0, n)
                e = tmp.tile([H, NN], bf, tag="tw", name="e")
                nc.scalar.activation(e[:], Th2[:, 1, :NN], AF.Exp,
                                     bias=bias_t[:, 0:1], scale=-1.0)
                d2 = tmp.tile([H, NN], bf, tag="tw", name="d2")
                nc.vector.tensor_tensor(
                    d2[:], xbt[:, BS(b0):BS(b0) + NN], Th2[:, 0, :NN],
                    OP.subtract)
                z = tmp.tile([H, NN], bf, tag="tw", name="z")
                nc.gpsimd.tensor_tensor(z[:], d2[:], e[:], OP.mult)
                z2 = tmp.tile([H, NN], bf, tag="tw", name="z2")
                nc.gpsimd.tensor_tensor(z2[:], z[:], z[:], OP.mult)
                for j in range(n):
                    b = b0 + j
                    scrap = tmp.tile([H, 128], bf, tag="tw", name="scr")
                    nc.vector.tensor_tensor_reduce(
                        scrap[:],
                        z2[:, j * 130 + 1:j * 130 + 129],
                        Th2[:, 1, j * 130 + 1:j * 130 + 129],
                        1.0, lp[:, b:b + 1], OP.add, OP.add,
                        lp[:, b:b + 1])

            # ---------------- cond phase ----------------
            with tc.tile_pool(name="otb", bufs=1) as otp, \
                 tc.tile_pool(name="cstr", bufs=2) as cstr:
                ot = otp.tile([H, n_ot, 2, H], f8, tag="ot", name="ot")
                for k in range(4):
                    s = (n_ot * k) // 4
                    e = (n_ot * (k + 1)) // 4
                    nc.sync.dma_start(
                        ot[:, s:e],
                        bdd[s:e].rearrange("n p t m -> p n t m"))

                def bot(key, j):
                    o, n = off[key]
                    assert j < n
                    return ot[:, o + j]

                tc8 = state.tile([H, 2, FREE], f8, tag="tc8", name="tc8")
                nc.vector.memset(tc8[:], 0.0)

                pc = {}
                for ci, (b0, n) in enumerate(CHUNKS):
                    pc[ci] = psum.tile([H, 2, 512], f32, tag="ps", name="ps")
                for k in range(8):
                    cpl = cstr.tile([H, 2, FREE], f8, tag="cpl", name="cpl")
                    nc.sync.dma_start(
                        cpl[:], cdd[k].rearrange("t h w -> h t w"))
                    for ci, (b0, n) in enumerate(CHUNKS):
                        for co in range(2):
                            key = f"c1_{co}"
                            tot = off[key][1]
                            for j in range(n):
                                base = BS(b0 + j)
                                out = pc[ci][:, co, j * 130:(j + 1) * 130]
                                for dxi, dx in enumerate((-1, 0, 1)):
                                    idx = k * 3 + dxi
                                    last = (idx == tot - 1)
                                    nc.tensor.matmul(
                                        out, bot(key, idx),
                                        cpl[:, :, base + dx:base + dx + 130],
                                        start=(idx == 0), stop=last,
                                        perf_mode=DR)
                                if k == 7 and _nz(bc1[co]):
                                    nc.tensor.matmul(
                                        out, bot(key, tot - 1),
                                        ones8[:, :, 0:130],
                                        start=False, stop=True, perf_mode=DR)
                for ci, (b0, n) in enumerate(CHUNKS):
                    NN = n * 130
                    nc.scalar.activation(
                        interior(tc8[:, :, BS(b0):BS(b0) + NN]),
                        interior(pc[ci][:, :, :NN]), AF.Tanh)
                # cond2 -> cf slots of P
                for ci, (b0, n) in enumerate(CHUNKS):
                    NN = n * 130
                    pq = psum.tile([H, 2, 512], f32, tag="ps", name="ps")
                    for co in range(2):
                        key = f"c2_{co}"
                        tot = off[key][1]
                        for j in range(n):
                            base = BS(b0 + j)
                            out = pq[:, co, j * 130:(j + 1) * 130]
                            for dxi, dx in enumerate((-1, 0, 1)):
                                nc.tensor.matmul(
                                    out, bot(key, dxi),
                                    tc8[:, :, base + dx:base + dx + 130],
                                    start=(dxi == 0), stop=(dxi == tot - 1),
                                    perf_mode=DR)
                            if _nz(bc2[co]):
                                nc.tensor.matmul(
                                    out, bot(key, tot - 1),
                                    ones8[:, :, 0:130],
                                    start=False, stop=True, perf_mode=DR)
                    nc.scalar.activation(
                        interior(P[:, 2:4, BS(b0):BS(b0) + NN]),
                        interior(pq[:, :, :NN]), AF.Identity)

            # ---------------- step 0 ----------------
            xbt0 = bstream.tile([H, FREE], bf, tag="xbt", name="xbt")
            nc.sync.dma_start(xbt0[:], xbd[0])
            h1b = (_nz(bo1[0]), _nz(bo1[1]))
            for (b0, n) in CHUNKS:
                NN = n * 130
                Th1 = psum.tile([H, 2, 512], f32, tag="ps", name="ps")
                head("h1", (0, 0), Th1, b0, n, include_h=False,
                     bias_flags=h1b)
                nc.vector.tensor_scalar(
                    interior(P[:, 4:6, BS(b0):BS(b0) + NN]),
                    interior(Th1[:, :, :NN]), 0.0, None, OP.max)
                head2_and_lp(b0, n, xbt0)

            # ---------------- steps (phase-major across chunks) ----------
            gb = [_nz(bih[g]) or _nz(bci[0]) for g in range(8)]
            for st in range(1, 16):
                xpl = xstream.tile([H, 2, FREE], f8, tag="xpl", name="xpl")
                nc.sync.dma_start(xpl[:], xd8[st - 1].rearrange(
                    "t h w -> h t w"))
                xbt = bstream.tile([H, FREE], bf, tag="xbt", name="xbt")
                nc.sync.dma_start(xbt[:], xbd[st])
                # Phase A: gate matmuls + tanh + sigmoids, all chunks
                tnh = {}
                sig = {}
                for ci, (b0, n) in enumerate(CHUNKS):
                    NN = n * 130
                    for g in range(4):
                        Tg = psum.tile([H, 2, 512], f32, tag="ps", name="ps")
                        for f01 in range(2):
                            co = 2 * g + f01
                            key = f"g{co}"
                            tot = off[key][1]
                            for j in range(n):
                                base = BS(b0 + j)
                                out = Tg[:, f01, j * 130:(j + 1) * 130]
                                idx = 0
                                for dx in (-1, 0, 1):
                                    nc.tensor.matmul(
                                        out, bp(key, idx),
                                        P[:, 0:2, base + dx:base + dx + 130],
                                        start=(idx == 0),
                                        stop=(idx == tot - 1), perf_mode=DR)
                                    idx += 1
                                for xb_ in (-2, 0, 2):
                                    nc.tensor.matmul(
                                        out, bp(key, idx),
                                        xpl[:, :, base + xb_:base + xb_ + 130],
                                        start=False, stop=(idx == tot - 1),
                                        perf_mode=DR)
                                    idx += 1
                                if gb[co]:
                                    nc.tensor.matmul(
                                        out, bp(key, idx),
                                        ones8[:, :, 0:130],
                                        start=False, stop=True, perf_mode=DR)
                        t = tmp.tile([H, 2, NN], bf, tag="tw", name="tnh")
                        nc.scalar.activation(
                            t[:], Tg[:, :, :NN], AF.Tanh,
                            scale=(1.0 if g == 2 else 0.5))
                        tnh[(ci, g)] = t
                        if g != 2:
                            s = tmp.tile([H, 2, NN], bf, tag="tw", name="sig")
                            nc.vector.tensor_scalar(
                                s[:], t[:], 0.5, 0.5, OP.mult, OP.add)
                            sig[(ci, g)] = s
                # Phase B: cell update + tanh-c + h write, all chunks
                for ci, (b0, n) in enumerate(CHUNKS):
                    NN = n * 130
                    tig = tmp.tile([H, 2, NN], bf, tag="tw", name="tig")
                    nc.vector.tensor_tensor(
                        tig[:], sig[(ci, 0)][:], tnh[(ci, 2)][:], OP.mult)
                    cs = cst_t[:, :, BS(b0):BS(b0) + NN]
                    nc.vector.tensor_tensor(cs, sig[(ci, 1)][:], cs, OP.mult)
                    nc.vector.tensor_tensor(cs, cs, tig[:], OP.add)
                    tcc = tmp.tile([H, 2, NN], bf, tag="tw", name="tcc")
                    nc.scalar.activation(tcc[:], cs, AF.Tanh)
                    nc.vector.tensor_tensor(
                        interior(P[:, 0:2, BS(b0):BS(b0) + NN]),
                        interior(sig[(ci, 3)][:]), interior(tcc[:]), OP.mult)
                # Phase C: head1 + relu, all chunks
                for ci, (b0, n) in enumerate(CHUNKS):
                    NN = n * 130
                    Th1 = psum.tile([H, 2, 512], f32, tag="ps", name="ps")
                    head("h1", (0, 0), Th1, b0, n, bias_flags=h1b)
                    nc.vector.tensor_scalar(
                        interior(P[:, 4:6, BS(b0):BS(b0) + NN]),
                        interior(Th1[:, :, :NN]), 0.0, None, OP.max)
                # Phase D: head2 + lp tail, all chunks
                for ci, (b0, n) in enumerate(CHUNKS):
                    head2_and_lp(b0, n, xbt)

            # final (reuse a main psum-pool tile for the [8,1] reduction)
            po_t = psum.tile([H, 2, 512], f32, tag="ps", name="ps")
            po = po_t[:BL, 0, 0:1]
            nc.tensor.matmul(po, lp[:], ones_f[:], start=True, stop=True)
            osb = state.tile([BL, 1], f32, tag="osb", name="osb")
            nc.scalar.activation(osb[:], po, AF.Identity,
                                 scale=-1.0, bias=bias_t[:BL, 1:2])
            nc.sync.dma_start(od[:], osb[:])
    nc.compile()
    return nc


def _host_inputs(inputs):
    x = np.ascontiguousarray(inputs["x"], np.float32)
    cond = np.ascontiguousarray(inputs["cond"], np.float32)
    bo2 = np.asarray(inputs["bo2"], np.float32)

    bands, off, NP = _build_bands(
        np.asarray(inputs["Wci"], np.float32),
        np.asarray(inputs["Wc1"], np.float32),
        np.asarray(inputs["Wc2"], np.float32),
        np.asarray(inputs["Wo1"], np.float32),
        np.asarray(inputs["Wo2"], np.float32),
        np.asarray(inputs["Wih"], np.float32),
        np.asarray(inputs["Whh"], np.float32),
        np.asarray(inputs["bci"], np.float32),
        np.asarray(inputs["bc1"], np.float32),
        np.asarray(inputs["bc2"], np.float32),
        np.asarray(inputs["bo1"], np.float32),
        bo2,
        np.asarray(inputs["bih"], np.float32))

    x8 = np.zeros((NCORES, C - 1, 2, H, FREE), F8)
    xb = np.zeros((NCORES, C, H, FREE), BF16)
    c8 = np.zeros((NCORES, 8, 2, H, FREE), F8)
    for core in range(NCORES):
        xs = x[core * BL:(core + 1) * BL]        # [8, C, H, W]
        cs = cond[core * BL:(core + 1) * BL]
        A = np.zeros((C, H, FREE), np.float32)
        Ac = np.zeros((C, H, FREE), np.float32)
        for b in range(BL):
            s = OFF + b * WB + 1
            A[:, :, s:s + 128] = xs[b]
            Ac[:, :, s:s + 128] = cs[b]
        A8 = A.astype(F8)
        x8[core, :, 0] = A8[:C - 1]
        x8[core, :C - 1, 1, :, :-1] = A8[:C - 1, :, 1:]
        xb[core] = (A - float(bo2[0])).astype(BF16)
        Ac8 = Ac.astype(F8)
        c8[core, :, 0] = Ac8[0::2]
        c8[core, :, 1] = Ac8[1::2]
    return x8, xb, c8, bands


def kernel(**inputs):
    x8, xb, c8, bands = _host_inputs(inputs)
    nc = _build_program(
        np.asarray(inputs["bci"], np.float32),
        np.asarray(inputs["bc1"], np.float32),
        np.asarray(inputs["bc2"], np.float32),
        np.asarray(inputs["bo1"], np.float32),
        np.asarray(inputs["bo2"], np.float32),
        np.asarray(inputs["bih"], np.float32))
    from concourse.bass_utils import run_bass_kernel_spmd
    in_maps = [
        {"x8": x8[i], "xb": xb[i], "c8": c8[i], "bands": bands}
        for i in range(NCORES)
    ]
    res = run_bass_kernel_spmd(nc, in_maps, list(range(NCORES)))
    out = np.concatenate(
        [res.results[i]["out"].reshape(BL) for i in range(NCORES)])
    return out.astype(np.float32)


if __name__ == "__main__":
    rng = np.random.default_rng(0)
    ins = {
        "x": rng.standard_normal((64, 16, 128, 128)).astype(np.float32),
        "cond": rng.standard_normal((64, 16, 128, 128)).astype(np.float32),
        "Wci": (rng.standard_normal((3, 3, 1, 1)) * 0.1).astype(np.float32),
        "bci": np.zeros(1, np.float32),
        "Wc1": (rng.standard_normal((3, 3, 16, 2)) * 0.1).astype(np.float32),
        "bc1": np.zeros(2, np.float32),
        "Wc2": (rng.standard_normal((3, 3, 2, 2)) * 0.1).astype(np.float32),
        "bc2": np.zeros(2, np.float32),
        "Wo1": (rng.standard_normal((3, 3, 4, 2)) * 0.1).astype(np.float32),
        "bo1": np.zeros(2, np.float32),
        "Wo2": (rng.standard_normal((3, 3, 2, 2)) * 0.1).astype(np.float32),
        "bo2": np.zeros(2, np.float32),
        "Wih": (rng.standard_normal((3, 3, 1, 8)) * 0.1).astype(np.float32),
        "bih": np.zeros(8, np.float32),
        "Whh": (rng.standard_normal((3, 3, 2, 8)) * 0.1).astype(np.float32),
    }
    print(kernel(**ins)[:8])
